# revision 14
# baseline (speedup 1.0000x reference)
"""GraphSAGE pool-aggregator kernel for 8 TRN2 NeuronCores.

reference:
    neighs = features[neigh_idx]              # [B, K, D] gather
    z = neighs @ W + b                        # [B, K, D]
    h = sigmoid(max(z, axis=1))               # [B, D]

Strategy: data-parallel over target nodes (shard neigh_idx 8 ways, replicate
features/W/b). Features are cast to bf16 host-side (halves gather traffic;
rel-err contribution ~3e-4). Per core:
  - indirect-DMA gather of 128 neighbor rows per op -> [128, D] bf16
    (SWDGE descriptor generation at ~8.7ns/row is the throughput wall)
  - PE transpose per 128-row tile -> PSUM bf16
  - DVE copy -> [D, 512] bf16 xT staging
  - one bf16 matmul  z.T = W.T @ xT  -> PSUM f32 [D, 512]
  - DVE segmented reduce_max over K=32 -> [D, 16]
  - ACT sigmoid(x + b) -> output staging [D, nodes]; DMA out in chunks
Output is produced transposed ([D, nodes_per_core]); host transposes back.
"""

import sys

for _p in ("/opt/trn_rl_repo",):
    if _p not in sys.path:
        sys.path.insert(0, _p)

import ml_dtypes
import numpy as np

import concourse.bass as bass
import concourse.tile as tile
from concourse import bacc
from concourse import mybir
from concourse.bass_utils import run_bass_kernel_spmd
from concourse.masks import make_identity

P = 128          # partitions / gather tile rows
D = 128          # feature dim (in == out)
K = 32           # neighbors per node
V = 1_000_000    # feature table rows
B = 50_000       # target nodes
N_CORES = 8

B_CORE = B // N_CORES              # 6250 real nodes per core
TILES_PER_GROUP = 4                # 4 x 128 gathered rows per matmul group
NODES_PER_GROUP = TILES_PER_GROUP * P // K   # 16 nodes per group
GROUPS_PER_CHUNK = 28              # output staging chunk: 448 nodes
B_CORE_PAD = 6272                  # padded to 392 groups (= 14 chunks)
N_GROUPS = B_CORE_PAD * K // (TILES_PER_GROUP * P)   # 392
N_TILES = N_GROUPS * TILES_PER_GROUP                 # 1568
GATHER_TILES = 16                  # tiles gathered per indirect DMA op

BF16 = mybir.dt.bfloat16
F32 = mybir.dt.float32


def build_nc(v=V, n_groups=N_GROUPS, groups_per_chunk=GROUPS_PER_CHUNK,
             gather_tiles=GATHER_TILES, debug=None):
    assert n_groups % groups_per_chunk == 0
    n_tiles = n_groups * TILES_PER_GROUP
    n_nodes = n_groups * NODES_PER_GROUP
    chunk_nodes = groups_per_chunk * NODES_PER_GROUP
    assert n_tiles % gather_tiles == 0
    assert gather_tiles % TILES_PER_GROUP == 0
    groups_per_gather = gather_tiles // TILES_PER_GROUP

    nc = bacc.Bacc()
    feat = nc.declare_dram_parameter("features", [v, D], BF16, isOutput=False)
    w = nc.declare_dram_parameter("w", [D, D], BF16, isOutput=False)
    bvec = nc.declare_dram_parameter("bias", [D, 1], F32, isOutput=False)
    idx = nc.declare_dram_parameter("idx", [P, n_tiles], mybir.dt.int32, isOutput=False)
    out = nc.declare_dram_parameter("out", [D, n_nodes], F32, isOutput=True)

    with tile.TileContext(nc) as tc:
        with (
            tc.tile_pool(name="singles", bufs=1) as singles,
            tc.tile_pool(name="gpool", bufs=8) as gpool,
            tc.tile_pool(name="xtpool", bufs=3) as xtpool,
            tc.tile_pool(name="redpool", bufs=4) as redpool,
            tc.tile_pool(name="outpool", bufs=2) as outpool,
            tc.tile_pool(name="pt", bufs=2, space="PSUM") as pt_pool,
            tc.tile_pool(name="pz", bufs=2, space="PSUM") as pz_pool,
        ):
            w_sb = singles.tile([D, D], BF16)
            nc.sync.dma_start(out=w_sb[:], in_=w[:])
            b_sb = singles.tile([D, 1], F32)
            nc.sync.dma_start(out=b_sb[:], in_=bvec[:])
            ident = singles.tile([P, P], BF16)
            make_identity(nc, ident[:])
            idx_sb = singles.tile([P, n_tiles], mybir.dt.int32)
            nc.sync.dma_start(out=idx_sb[:], in_=idx[:])

            for chunk in range(n_groups // groups_per_chunk):
                stage = outpool.tile([D, chunk_nodes], F32)
                for gi in range(groups_per_chunk):
                    g = chunk * groups_per_chunk + gi
                    xt = xtpool.tile([D, TILES_PER_GROUP * P], BF16)
                    ptile = pt_pool.tile([P, TILES_PER_GROUP * P], BF16)
                    gt = gpool.tile([P, TILES_PER_GROUP * D], BF16, tag="g")
                    for j in range(TILES_PER_GROUP):
                        t = g * TILES_PER_GROUP + j
                        nc.gpsimd.indirect_dma_start(
                            out=gt[:, j * D : (j + 1) * D],
                            out_offset=None,
                            in_=feat[:],
                            in_offset=bass.IndirectOffsetOnAxis(
                                ap=idx_sb[:, t : t + 1], axis=0
                            ),
                        )
                    for j in range(TILES_PER_GROUP):
                        nc.tensor.transpose(
                            out=ptile[:, j * P : (j + 1) * P],
                            in_=gt[:, j * D : (j + 1) * D],
                            identity=ident[:],
                        )
                    nc.vector.tensor_copy(out=xt[:], in_=ptile[:])
                    z = pz_pool.tile([P, TILES_PER_GROUP * P], F32)
                    nc.tensor.matmul(
                        out=z[:], lhsT=w_sb[:], rhs=xt[:], start=True, stop=True
                    )
                    red = redpool.tile([P, NODES_PER_GROUP], F32)
                    nc.vector.reduce_max(
                        out=red[:],
                        in_=z[:].rearrange("p (n k) -> p n k", k=K),
                        axis=mybir.AxisListType.X,
                    )
                    nc.scalar.activation(
                        out=stage[:, gi * NODES_PER_GROUP : (gi + 1) * NODES_PER_GROUP],
                        in_=red[:],
                        func=mybir.ActivationFunctionType.Sigmoid,
                        bias=b_sb[:, :1],
                        scale=1.0,
                    )
                nc.sync.dma_start(
                    out=out[:, chunk * chunk_nodes : (chunk + 1) * chunk_nodes],
                    in_=stage[:],
                )
    nc.compile()
    return nc


def make_in_maps(features, W, b, neigh_idx, b_core=B_CORE, b_core_pad=B_CORE_PAD):
    """Shard host inputs for the 8 cores."""
    features = np.asarray(features, dtype=np.float32).astype(ml_dtypes.bfloat16)
    W = np.asarray(W, dtype=np.float32).astype(ml_dtypes.bfloat16)
    b_col = np.ascontiguousarray(np.asarray(b, dtype=np.float32)).reshape(D, 1)
    idx32 = np.asarray(neigh_idx).astype(np.int32)
    n_tiles = b_core_pad * K // P

    in_maps = []
    for c in range(N_CORES):
        sl = idx32[c * b_core : (c + 1) * b_core]            # [b_core, K]
        if b_core_pad > b_core:
            pad = np.zeros((b_core_pad - b_core, K), np.int32)
            sl = np.concatenate([sl, pad], axis=0)
        flat = sl.reshape(-1)                                # node-major
        idx_t = np.ascontiguousarray(flat.reshape(n_tiles, P).T)  # [P, n_tiles]
        in_maps.append(
            {
                "features": features,
                "w": W,
                "bias": b_col,
                "idx": idx_t,
            }
        )
    return in_maps


def kernel(features, W, b, neigh_idx, trace=False):
    in_maps = make_in_maps(features, W, b, neigh_idx)
    nc = build_nc()
    res = run_bass_kernel_spmd(nc, in_maps, core_ids=list(range(N_CORES)), trace=trace)
    outs = [m["out"] for m in res.results]                   # [D, B_CORE_PAD] each
    h = np.concatenate([o[:, :B_CORE].T for o in outs], axis=0)
    if trace:
        kernel.last_exec_time_ns = res.exec_time_ns
    return np.ascontiguousarray(h.astype(np.float32))


kernel.last_exec_time_ns = None


# revision 15
# speedup vs baseline: 1.1785x; 1.1785x over previous
"""GraphSAGE pool-aggregator kernel for 8 TRN2 NeuronCores.

reference:
    neighs = features[neigh_idx]              # [B, K, D] gather
    z = neighs @ W + b                        # [B, K, D]
    h = sigmoid(max(z, axis=1))               # [B, D]

Strategy: data-parallel over target nodes (shard neigh_idx 8 ways, replicate
features/W/b). Features are cast to bf16 host-side (halves gather traffic;
rel-err contribution ~3e-4). Per core:
  - indirect-DMA gather of 128 neighbor rows per op -> [128, D] bf16
    (SWDGE descriptor generation at ~8.7ns/row is the throughput wall)
  - PE transpose per 128-row tile -> PSUM bf16
  - DVE copy -> [D, 512] bf16 xT staging
  - one bf16 matmul  z.T = W.T @ xT  -> PSUM f32 [D, 512]
  - DVE segmented reduce_max over K=32 -> [D, 16]
  - ACT sigmoid(x + b) -> output staging [D, nodes]; DMA out in chunks
Output is produced transposed ([D, nodes_per_core]); host transposes back.
"""

import sys

for _p in ("/opt/trn_rl_repo",):
    if _p not in sys.path:
        sys.path.insert(0, _p)

import ml_dtypes
import numpy as np

import concourse.bass as bass
import concourse.tile as tile
from concourse import bacc
from concourse import mybir
from concourse.bass_utils import run_bass_kernel_spmd
from concourse.masks import make_identity

P = 128          # partitions / gather tile rows
D = 128          # feature dim (in == out)
K = 32           # neighbors per node
V = 1_000_000    # feature table rows
B = 50_000       # target nodes
N_CORES = 8

B_CORE = B // N_CORES              # 6250 real nodes per core
TILES_PER_GROUP = 4                # 4 x 128 gathered rows per matmul group
NODES_PER_GROUP = TILES_PER_GROUP * P // K   # 16 nodes per group
GROUPS_PER_CHUNK = 28              # output staging chunk: 448 nodes
B_CORE_PAD = 6272                  # padded to 392 groups (= 14 chunks)
N_GROUPS = B_CORE_PAD * K // (TILES_PER_GROUP * P)   # 392
N_TILES = N_GROUPS * TILES_PER_GROUP                 # 1568
GATHER_TILES = 16                  # tiles gathered per indirect DMA op

BF16 = mybir.dt.bfloat16
F32 = mybir.dt.float32


def build_nc(v=V, n_groups=N_GROUPS, groups_per_chunk=GROUPS_PER_CHUNK,
             gather_tiles=GATHER_TILES, debug=None):
    assert n_groups % groups_per_chunk == 0
    n_tiles = n_groups * TILES_PER_GROUP
    n_nodes = n_groups * NODES_PER_GROUP
    chunk_nodes = groups_per_chunk * NODES_PER_GROUP
    assert n_tiles % gather_tiles == 0
    assert gather_tiles % TILES_PER_GROUP == 0
    groups_per_gather = gather_tiles // TILES_PER_GROUP

    nc = bacc.Bacc()
    feat = nc.declare_dram_parameter("features", [v, D], BF16, isOutput=False)
    w = nc.declare_dram_parameter("w", [D, D], BF16, isOutput=False)
    bvec = nc.declare_dram_parameter("bias", [D, 1], F32, isOutput=False)
    idx = nc.declare_dram_parameter("idx", [P, n_tiles], mybir.dt.int32, isOutput=False)
    out = nc.declare_dram_parameter("out", [D, n_nodes], F32, isOutput=True)

    with tile.TileContext(nc) as tc:
        with (
            tc.tile_pool(name="singles", bufs=1) as singles,
            tc.tile_pool(name="gpool", bufs=12) as gpool,
            tc.tile_pool(name="xtpool", bufs=3) as xtpool,
            tc.tile_pool(name="redpool", bufs=4) as redpool,
            tc.tile_pool(name="outpool", bufs=2) as outpool,
            tc.tile_pool(name="pt", bufs=2, space="PSUM") as pt_pool,
            tc.tile_pool(name="pz", bufs=2, space="PSUM") as pz_pool,
        ):
            w_sb = singles.tile([D, D], BF16)
            nc.sync.dma_start(out=w_sb[:], in_=w[:])
            b_sb = singles.tile([D, 1], F32)
            nc.sync.dma_start(out=b_sb[:], in_=bvec[:])
            ident = singles.tile([P, P], BF16)
            make_identity(nc, ident[:])
            idx_sb = singles.tile([P, n_tiles], mybir.dt.int32)
            nc.sync.dma_start(out=idx_sb[:], in_=idx[:])

            for chunk in range(n_groups // groups_per_chunk):
                stage = outpool.tile([D, chunk_nodes], F32)
                for gi in range(groups_per_chunk):
                    g = chunk * groups_per_chunk + gi
                    xt = xtpool.tile([D, TILES_PER_GROUP * P], BF16)
                    ptile = pt_pool.tile([P, TILES_PER_GROUP * P], BF16)
                    for j in range(TILES_PER_GROUP):
                        t = g * TILES_PER_GROUP + j
                        gt = gpool.tile([P, D], BF16, tag="g")
                        nc.gpsimd.indirect_dma_start(
                            out=gt[:],
                            out_offset=None,
                            in_=feat[:],
                            in_offset=bass.IndirectOffsetOnAxis(
                                ap=idx_sb[:, t : t + 1], axis=0
                            ),
                        )
                        nc.tensor.transpose(
                            out=ptile[:, j * P : (j + 1) * P],
                            in_=gt[:],
                            identity=ident[:],
                        )
                    nc.vector.tensor_copy(out=xt[:], in_=ptile[:])
                    z = pz_pool.tile([P, TILES_PER_GROUP * P], F32)
                    nc.tensor.matmul(
                        out=z[:], lhsT=w_sb[:], rhs=xt[:], start=True, stop=True
                    )
                    red = redpool.tile([P, NODES_PER_GROUP], F32)
                    nc.vector.reduce_max(
                        out=red[:],
                        in_=z[:].rearrange("p (n k) -> p n k", k=K),
                        axis=mybir.AxisListType.X,
                    )
                    nc.scalar.activation(
                        out=stage[:, gi * NODES_PER_GROUP : (gi + 1) * NODES_PER_GROUP],
                        in_=red[:],
                        func=mybir.ActivationFunctionType.Sigmoid,
                        bias=b_sb[:, :1],
                        scale=1.0,
                    )
                nc.sync.dma_start(
                    out=out[:, chunk * chunk_nodes : (chunk + 1) * chunk_nodes],
                    in_=stage[:],
                )
    nc.compile()
    return nc


def make_in_maps(features, W, b, neigh_idx, b_core=B_CORE, b_core_pad=B_CORE_PAD):
    """Shard host inputs for the 8 cores."""
    features = np.asarray(features, dtype=np.float32).astype(ml_dtypes.bfloat16)
    W = np.asarray(W, dtype=np.float32).astype(ml_dtypes.bfloat16)
    b_col = np.ascontiguousarray(np.asarray(b, dtype=np.float32)).reshape(D, 1)
    idx32 = np.asarray(neigh_idx).astype(np.int32)
    n_tiles = b_core_pad * K // P

    in_maps = []
    for c in range(N_CORES):
        sl = idx32[c * b_core : (c + 1) * b_core]            # [b_core, K]
        if b_core_pad > b_core:
            pad = np.zeros((b_core_pad - b_core, K), np.int32)
            sl = np.concatenate([sl, pad], axis=0)
        flat = sl.reshape(-1)                                # node-major
        idx_t = np.ascontiguousarray(flat.reshape(n_tiles, P).T)  # [P, n_tiles]
        in_maps.append(
            {
                "features": features,
                "w": W,
                "bias": b_col,
                "idx": idx_t,
            }
        )
    return in_maps


def kernel(features, W, b, neigh_idx, trace=False):
    in_maps = make_in_maps(features, W, b, neigh_idx)
    nc = build_nc()
    res = run_bass_kernel_spmd(nc, in_maps, core_ids=list(range(N_CORES)), trace=trace)
    outs = [m["out"] for m in res.results]                   # [D, B_CORE_PAD] each
    h = np.concatenate([o[:, :B_CORE].T for o in outs], axis=0)
    if trace:
        kernel.last_exec_time_ns = res.exec_time_ns
    return np.ascontiguousarray(h.astype(np.float32))


kernel.last_exec_time_ns = None


# revision 16
# speedup vs baseline: 1.1822x; 1.0031x over previous
"""GraphSAGE pool-aggregator kernel for 8 TRN2 NeuronCores.

reference:
    neighs = features[neigh_idx]              # [B, K, D] gather
    z = neighs @ W + b                        # [B, K, D]
    h = sigmoid(max(z, axis=1))               # [B, D]

Strategy: data-parallel over target nodes (shard neigh_idx 8 ways, replicate
features/W/b). Features are cast to bf16 host-side (halves gather traffic;
rel-err contribution ~3e-4). Per core:
  - indirect-DMA gather of 128 neighbor rows per op -> [128, D] bf16
    (SWDGE descriptor generation at ~8.7ns/row is the throughput wall)
  - PE transpose per 128-row tile -> PSUM bf16
  - DVE copy -> [D, 512] bf16 xT staging
  - one bf16 matmul  z.T = W.T @ xT  -> PSUM f32 [D, 512]
  - DVE segmented reduce_max over K=32 -> [D, 16]
  - ACT sigmoid(x + b) -> output staging [D, nodes]; DMA out in chunks
Output is produced transposed ([D, nodes_per_core]); host transposes back.
"""

import sys

for _p in ("/opt/trn_rl_repo",):
    if _p not in sys.path:
        sys.path.insert(0, _p)

import ml_dtypes
import numpy as np

import concourse.bass as bass
import concourse.tile as tile
from concourse import bacc
from concourse import mybir
from concourse.bass_utils import run_bass_kernel_spmd
from concourse.masks import make_identity

P = 128          # partitions / gather tile rows
D = 128          # feature dim (in == out)
K = 32           # neighbors per node
V = 1_000_000    # feature table rows
B = 50_000       # target nodes
N_CORES = 8

B_CORE = B // N_CORES              # 6250 real nodes per core
TILES_PER_GROUP = 4                # 4 x 128 gathered rows per matmul group
NODES_PER_GROUP = TILES_PER_GROUP * P // K   # 16 nodes per group
GROUPS_PER_CHUNK = 28              # output staging chunk: 448 nodes
B_CORE_PAD = 6272                  # padded to 392 groups (= 14 chunks)
N_GROUPS = B_CORE_PAD * K // (TILES_PER_GROUP * P)   # 392
N_TILES = N_GROUPS * TILES_PER_GROUP                 # 1568
GATHER_TILES = 16                  # tiles gathered per indirect DMA op

BF16 = mybir.dt.bfloat16
F32 = mybir.dt.float32


def build_nc(v=V, n_groups=N_GROUPS, groups_per_chunk=GROUPS_PER_CHUNK,
             gather_tiles=GATHER_TILES, debug=None):
    assert n_groups % groups_per_chunk == 0
    n_tiles = n_groups * TILES_PER_GROUP
    n_nodes = n_groups * NODES_PER_GROUP
    chunk_nodes = groups_per_chunk * NODES_PER_GROUP
    assert n_tiles % gather_tiles == 0
    assert gather_tiles % TILES_PER_GROUP == 0
    groups_per_gather = gather_tiles // TILES_PER_GROUP

    nc = bacc.Bacc()
    feat = nc.declare_dram_parameter("features", [v, D], BF16, isOutput=False)
    w = nc.declare_dram_parameter("w", [D, D], BF16, isOutput=False)
    bvec = nc.declare_dram_parameter("bias", [D, 1], F32, isOutput=False)
    idx = nc.declare_dram_parameter("idx", [P, n_tiles], mybir.dt.int32, isOutput=False)
    out = nc.declare_dram_parameter("out", [D, n_nodes], F32, isOutput=True)

    with tile.TileContext(nc) as tc:
        with (
            tc.tile_pool(name="singles", bufs=1) as singles,
            tc.tile_pool(name="gpool", bufs=6) as gpool,
            tc.tile_pool(name="xtpool", bufs=3) as xtpool,
            tc.tile_pool(name="redpool", bufs=4) as redpool,
            tc.tile_pool(name="outpool", bufs=2) as outpool,
            tc.tile_pool(name="pt", bufs=2, space="PSUM") as pt_pool,
            tc.tile_pool(name="pz", bufs=2, space="PSUM") as pz_pool,
        ):
            w_sb = singles.tile([D, D], BF16)
            nc.sync.dma_start(out=w_sb[:], in_=w[:])
            b_sb = singles.tile([D, 1], F32)
            nc.sync.dma_start(out=b_sb[:], in_=bvec[:])
            ident = singles.tile([P, P], BF16)
            make_identity(nc, ident[:])
            idx_sb = singles.tile([P, n_tiles], mybir.dt.int32)
            nc.sync.dma_start(out=idx_sb[:], in_=idx[:])

            for chunk in range(n_groups // groups_per_chunk):
                stage = outpool.tile([D, chunk_nodes], F32)
                for gi in range(groups_per_chunk):
                    g = chunk * groups_per_chunk + gi
                    xt = xtpool.tile([D, TILES_PER_GROUP * P], BF16)
                    ptile = pt_pool.tile([P, TILES_PER_GROUP * P], BF16)
                    gt = gpool.tile([P, TILES_PER_GROUP * D], BF16, tag="g")
                    for j in range(TILES_PER_GROUP):
                        t = g * TILES_PER_GROUP + j
                        nc.gpsimd.indirect_dma_start(
                            out=gt[:, j * D : (j + 1) * D],
                            out_offset=None,
                            in_=feat[:],
                            in_offset=bass.IndirectOffsetOnAxis(
                                ap=idx_sb[:, t : t + 1], axis=0
                            ),
                        )
                        nc.tensor.transpose(
                            out=ptile[:, j * P : (j + 1) * P],
                            in_=gt[:, j * D : (j + 1) * D],
                            identity=ident[:],
                        )
                    nc.vector.tensor_copy(out=xt[:], in_=ptile[:])
                    z = pz_pool.tile([P, TILES_PER_GROUP * P], F32)
                    nc.tensor.matmul(
                        out=z[:], lhsT=w_sb[:], rhs=xt[:], start=True, stop=True
                    )
                    red = redpool.tile([P, NODES_PER_GROUP], F32)
                    nc.vector.reduce_max(
                        out=red[:],
                        in_=z[:].rearrange("p (n k) -> p n k", k=K),
                        axis=mybir.AxisListType.X,
                    )
                    nc.scalar.activation(
                        out=stage[:, gi * NODES_PER_GROUP : (gi + 1) * NODES_PER_GROUP],
                        in_=red[:],
                        func=mybir.ActivationFunctionType.Sigmoid,
                        bias=b_sb[:, :1],
                        scale=1.0,
                    )
                nc.sync.dma_start(
                    out=out[:, chunk * chunk_nodes : (chunk + 1) * chunk_nodes],
                    in_=stage[:],
                )
    nc.compile()
    return nc


def make_in_maps(features, W, b, neigh_idx, b_core=B_CORE, b_core_pad=B_CORE_PAD):
    """Shard host inputs for the 8 cores."""
    features = np.asarray(features, dtype=np.float32).astype(ml_dtypes.bfloat16)
    W = np.asarray(W, dtype=np.float32).astype(ml_dtypes.bfloat16)
    b_col = np.ascontiguousarray(np.asarray(b, dtype=np.float32)).reshape(D, 1)
    idx32 = np.asarray(neigh_idx).astype(np.int32)
    n_tiles = b_core_pad * K // P

    in_maps = []
    for c in range(N_CORES):
        sl = idx32[c * b_core : (c + 1) * b_core]            # [b_core, K]
        if b_core_pad > b_core:
            pad = np.zeros((b_core_pad - b_core, K), np.int32)
            sl = np.concatenate([sl, pad], axis=0)
        flat = sl.reshape(-1)                                # node-major
        idx_t = np.ascontiguousarray(flat.reshape(n_tiles, P).T)  # [P, n_tiles]
        in_maps.append(
            {
                "features": features,
                "w": W,
                "bias": b_col,
                "idx": idx_t,
            }
        )
    return in_maps


def kernel(features, W, b, neigh_idx, trace=False):
    in_maps = make_in_maps(features, W, b, neigh_idx)
    nc = build_nc()
    res = run_bass_kernel_spmd(nc, in_maps, core_ids=list(range(N_CORES)), trace=trace)
    outs = [m["out"] for m in res.results]                   # [D, B_CORE_PAD] each
    h = np.concatenate([o[:, :B_CORE].T for o in outs], axis=0)
    if trace:
        kernel.last_exec_time_ns = res.exec_time_ns
    return np.ascontiguousarray(h.astype(np.float32))


kernel.last_exec_time_ns = None
